# revision 29
# baseline (speedup 1.0000x reference)
"""DN4 retrieval-KNN kernel for Trainium2 (8 NeuronCores, SPMD).

Computation (per episode batch b):
  sup   = mean_k support[b]  -> (5, 64, 441)           (class prototypes, local descriptors)
  logits[q, w] = sum_m max_n <qn[q,:,m], sn[w,:,n]>    (cosine sims of l2-normalized descriptors)

Sharding: 4 cores per batch element, 19 queries per core (75 = 19+19+19+18, last
core padded).  Support is replicated per batch-group; no cross-core comms.

v2 drain architecture: the PSUM->engine drain is the roofline (only DVE and ACT
can read PSUM, ~1 elem/cycle each).  The 20 per-query (m-chunk, class) sim
tiles stream through one triple-buffered 2-bank PSUM pool (PE runs ~6 matmuls
ahead of the drains) and split between two full-reduction pipes:
  - DVE reduce_max directly from PSUM (10 units): classes {0,1} all chunks +
    class 2 on chunks {0,1}; raw sims, the per-row 1/||q|| is applied by the
    logit matmul rhs.  Adjacent same-chunk units merge into one reduce.
  - ACT exp-accumulate LSE (10 units): classes {3,4} + class 2 on chunks
    {2,3}: S = sum_n exp(t*(cos-b)) via in-place activation(Exp, scale =
    t/||q|| per-partition AP, bias=-t*b, accum_out); maxv ~= b + ln(S)/t
    (t=128, b=0.35 keeps S inside the HW ln's accurate range >1.2e-20;
    rel err ~5e-3 vs the 2e-2 gate).
Other structural points: one manually preloaded ACT table set (id 6 has
Exp+Ln+Square+Copy) avoids per-func table thrashing; all rsqrt-s are
exp(-0.5*ln(x)); query ssq comes from a DVE square + PE ones-matmul into the
shared misc bank (no PSUM transposes); logits accumulate in three psum-base-0
column groups, remapped to class order by two end-of-kernel PE transposes so
the output DMA writes contiguous rows; invq/mask/maxv are bf16 so logit
matmuls are single-pass.
"""

import numpy as np

import concourse.bacc as bacc
import concourse.bass as bass
import concourse.mybir as mybir
import concourse.tile as tile
from concourse.bass_utils import run_bass_kernel_spmd

F32 = mybir.dt.float32
BF16 = mybir.dt.bfloat16
AX = mybir.AxisListType
ALU = mybir.AluOpType
ACT_EXP = mybir.ActivationFunctionType.Exp
ACT_LN = mybir.ActivationFunctionType.Ln
ACT_SQ = mybir.ActivationFunctionType.Square

B, NWAY, KSHOT, Q, C, HW = 2, 5, 5, 75, 64, 441  # 21*21 = 441
QPC = 19          # queries per core (8 cores: 4 per batch, 19/19/19/18+pad)
PADW = 512        # query free dim padded so m-chunks are 4x128 exactly
NCHUNK = 4
EPS = 1e-6        # ln(ssq+eps); pad rows get invq = 1e3, maxv = 0 there
T = 128.0         # LSE sharpness (on cosine scale)
LSEB = 0.35       # LSE center: S = sum exp(T*(cos-LSEB))
# (j, w) pairs drained via ACT-LSE; the rest go to DVE reduce_max.
LSE_JW = [(0, 3), (0, 4), (1, 3), (1, 4), (2, 3), (2, 4), (3, 3), (3, 4),
          (2, 2), (3, 2)]

_CACHE = {}


def _chunk_cols(j):
    lo = j * 128
    hi = min(lo + 128, HW)
    return lo, hi


def _build_program():
    nc = bacc.Bacc("TRN2", target_bir_lowering=False, debug=False, num_devices=8)

    sup_d = nc.dram_tensor("sup", [NWAY * KSHOT, C, HW], F32, kind="ExternalInput").ap()
    qry_d = nc.dram_tensor("qry", [QPC, C, HW], F32, kind="ExternalInput").ap()
    idn_d = nc.dram_tensor("idn", [128, 128], F32, kind="ExternalInput").ap()
    msk_d = nc.dram_tensor("msk", [128, NCHUNK], F32, kind="ExternalInput").ap()
    out_d = nc.dram_tensor("out", [QPC, NWAY], F32, kind="ExternalOutput").ap()

    with tile.TileContext(nc) as tc:
        with tc.tile_pool(name="const", bufs=1) as cpool:
            ident = cpool.tile([128, 128], F32)
            nc.sync.dma_start(ident[:], idn_d[:])
            identb = cpool.tile([128, 128], BF16)
            nc.vector.tensor_copy(identb[:], ident[:])
            mask = cpool.tile([128, NCHUNK], F32)
            nc.sync.dma_start(mask[:], msk_d[:])
            maskb = cpool.tile([128, NCHUNK], BF16)
            nc.vector.tensor_copy(maskb[:], mask[:])
            eps = cpool.tile([128, 1], F32)
            nc.vector.memset(eps[:], EPS)
            nbias = cpool.tile([128, 1], F32)
            nc.vector.memset(nbias[:], -T * LSEB)
            lnt = cpool.tile([128, 1], F32)
            nc.vector.memset(lnt[:], float(np.log(T)))
            zero = cpool.tile([128, 1], F32)
            nc.vector.memset(zero[:], 0.0)
            onesb = cpool.tile([C, 1], BF16)
            nc.vector.memset(onesb[:], 1.0)
            # preload the one table set covering Exp+Ln+Square+Copy so the
            # compiler's per-func greedy chooser doesn't thrash sets (id 6 =
            # natural_log_exp_and_others; engine FIFO keeps it first)
            nc.scalar.add_instruction(
                mybir.InstLoadActFuncSet(
                    name=nc.get_next_instruction_name(),
                    act_func_set_id=6,
                    ins=[],
                    outs=[],
                )
            )
            sn = [cpool.tile([128, PADW], BF16, name=f"sn{w}") for w in range(NWAY)]
            # stage3[p, g, i]: group g logits for query i (p = within-group row)
            stage3 = cpool.tile([2, 3, QPC], F32)

            # ---------------- support prototypes ----------------
            with (
                tc.tile_pool(name="sup_sb", bufs=3) as spool,
                tc.tile_pool(name="sup_ps", bufs=3, space="PSUM") as sps,
            ):
                for w in range(NWAY):
                    # shot sum via SWDGE accumulating DMA (mean/KSHOT cancels
                    # under the l2 normalization)
                    s5 = spool.tile([C, KSHOT * HW], F32, tag="s5")
                    nc.sync.dma_start(
                        s5[:].rearrange("c (k m) -> c k m", k=KSHOT),
                        sup_d[w * KSHOT : (w + 1) * KSHOT].rearrange("k c m -> c k m"),
                    )
                    # transpose all 4 chunks into one PSUM bank: (m, c),
                    # accumulating the shot sum on the PE
                    sT4 = sps.tile([128, NCHUNK, C], F32, tag="sT4")
                    for j in range(NCHUNK):
                        lo, hi = _chunk_cols(j)
                        for k in range(KSHOT):
                            nc.tensor.matmul(
                                sT4[0 : hi - lo, j, :],
                                lhsT=s5[:, k * HW + lo : k * HW + hi],
                                rhs=ident[0:C, 0:C],
                                is_transpose=True,
                                start=(k == 0),
                                stop=(k == KSHOT - 1),
                            )
                    # per-descriptor rsqrt(sum sq) via Square + ln/exp
                    sqs = spool.tile([128, NCHUNK * C], F32, tag="sqs")
                    nc.scalar.activation(
                        sqs[:], sT4[:].rearrange("p j c -> p (j c)"), ACT_SQ, bias=zero[:]
                    )
                    ssqs = spool.tile([128, NCHUNK], F32, tag="ssqs")
                    nc.vector.reduce_sum(
                        ssqs[:], sqs[:].rearrange("p (j c) -> p j c", j=NCHUNK), axis=AX.X
                    )
                    lns = spool.tile([128, NCHUNK], F32, tag="lns")
                    nc.scalar.activation(lns[:], ssqs[:], ACT_LN, bias=eps[:])
                    invs = spool.tile([128, NCHUNK], F32, tag="invs")
                    nc.scalar.activation(invs[:], lns[:], ACT_EXP, scale=-0.5, bias=zero[:])
                    snT4 = spool.tile([128, NCHUNK, C], BF16, tag="snT4")
                    for j in range(NCHUNK):
                        lo, hi = _chunk_cols(j)
                        nc.vector.tensor_scalar_mul(
                            snT4[0 : hi - lo, j, :],
                            sT4[0 : hi - lo, j, :],
                            invs[0 : hi - lo, j : j + 1],
                        )
                    # transpose back to (c, m): 4 chunks -> one psum tile
                    snb4 = sps.tile([C, NCHUNK, 128], BF16, tag="snb4")
                    for j in range(NCHUNK):
                        lo, hi = _chunk_cols(j)
                        nc.tensor.matmul(
                            snb4[:, j, 0 : hi - lo],
                            lhsT=snT4[0 : hi - lo, j, :],
                            rhs=identb[0 : hi - lo, 0 : hi - lo],
                            is_transpose=True,
                            start=True,
                            stop=True,
                        )
                    nc.vector.memset(sn[w][:, HW:PADW], 0.0)
                    # chunk stride in snb4 is 128 = chunk width, so the valid
                    # 441 columns are contiguous in the flattened view
                    nc.scalar.copy(
                        sn[w][0:C, 0:HW],
                        snb4[:].rearrange("c j m -> c (j m)")[:, 0:HW],
                    )
                    nc.sync.dma_start(sn[w][C:128, 0:HW], sn[w][0:C, 0:HW])

            # ---------------- queries ----------------
            # Sim tiles stream through one triple-buffered 2-bank pool so the
            # PE can run ~6 matmuls ahead of the DVE/ACT drains (keeps HAM
            # un-throttled).  Unit consumers: classes {0,1} + class 2 on
            # chunks {0,1} -> DVE reduce_max; the LSE_JW units -> ACT exp.
            with (
                tc.tile_pool(name="q_sb", bufs=4) as qpool,
                tc.tile_pool(name="qb_sb", bufs=4) as qbpool,
                tc.tile_pool(name="q_small", bufs=4) as qsm,
                tc.tile_pool(name="sim_ps", bufs=3, space="PSUM") as simps,
                tc.tile_pool(name="misc_ps", bufs=2, space="PSUM") as miscps,
            ):
                for i in range(QPC):
                    q2 = qpool.tile([C, PADW], F32, tag="q2")
                    nc.vector.memset(q2[:, HW:PADW], 0.0)
                    nc.sync.dma_start(q2[:, 0:HW], qry_d[i])
                    qb = qbpool.tile([128, PADW], BF16, tag="qb")
                    nc.gpsimd.dma_start(qb[0:C, :], q2[:])  # SWDGE cast f32->bf16
                    nc.sync.dma_start(qb[C:128, :], qb[0:C, :])

                    # 1/||q_m||: square on DVE, per-chunk column sums via a
                    # ones-matmul into the misc psum bank (cols 0:4; cols 4:7
                    # hold the three logit accumulator groups).
                    q2sq = qpool.tile([C, PADW], BF16, tag="q2sq")
                    nc.vector.tensor_mul(q2sq[:], q2[:], q2[:])
                    misc = miscps.tile([128, NCHUNK + 3], F32, tag="misc")
                    for j in range(NCHUNK):
                        nc.tensor.matmul(
                            misc[:, j : j + 1],
                            lhsT=q2sq[:, j * 128 : (j + 1) * 128],
                            rhs=onesb[0:C, :],
                            start=True,
                            stop=True,
                        )
                    lnq = qsm.tile([128, NCHUNK], F32, tag="lnq")
                    nc.scalar.activation(lnq[:], misc[:, 0:NCHUNK], ACT_LN, bias=eps[:])
                    inv = qsm.tile([128, NCHUNK], BF16, tag="inv")
                    nc.scalar.activation(inv[:], lnq[:], ACT_EXP, scale=-0.5, bias=zero[:])
                    tiv = qsm.tile([128, NCHUNK], F32, tag="tiv")
                    nc.scalar.activation(
                        tiv[:], lnq[:], ACT_EXP, scale=-0.5, bias=lnt[:]
                    )

                    maxv = qsm.tile([128, NCHUNK, NWAY], BF16, tag="maxv")
                    S = qsm.tile([128, len(LSE_JW)], F32, tag="S")
                    # flat (chunk, class) stream paired into 2-slice psum gens
                    units = [(j, w) for j in range(NCHUNK) for w in range(NWAY)]
                    lse_set = set(LSE_JW)
                    for g in range(len(units) // 2):
                        pair = units[2 * g : 2 * g + 2]
                        sim = simps.tile([128, 2, PADW], F32, tag="sim")
                        for s, (j, w) in enumerate(pair):
                            base = C * (w % 2)
                            nc.tensor.matmul(
                                sim[:, s, 0:HW],
                                lhsT=qb[base : base + C, j * 128 : (j + 1) * 128],
                                rhs=sn[w][base : base + C, 0:HW],
                                start=True,
                                stop=True,
                                tile_position=(base, 0),
                            )
                        # drain the two slices
                        if all((j, w) not in lse_set for (j, w) in pair) and (
                            pair[0][0] == pair[1][0] and pair[1][1] == pair[0][1] + 1
                        ):
                            j, w0 = pair[0]
                            nc.vector.reduce_max(
                                maxv[:, j, w0 : w0 + 2], sim[:, :, 0:HW], axis=AX.X
                            )
                        else:
                            for s, (j, w) in enumerate(pair):
                                if (j, w) in lse_set:
                                    col = LSE_JW.index((j, w))
                                    nc.scalar.activation(
                                        sim[:, s, 0:HW],
                                        sim[:, s, 0:HW],
                                        ACT_EXP,
                                        scale=tiv[:, j : j + 1],
                                        bias=nbias[:],
                                        accum_out=S[:, col : col + 1],
                                    )
                                else:
                                    nc.vector.reduce_max(
                                        maxv[:, j, w : w + 1], sim[:, s, 0:HW], axis=AX.X
                                    )

                    # LSE post: maxv = ln(S)/T + LSEB for the 10 LSE columns
                    lnS = qsm.tile([128, len(LSE_JW)], F32, tag="lnS")
                    nc.scalar.activation(lnS[:], S[:], ACT_LN, bias=zero[:])
                    nc.vector.tensor_scalar(
                        maxv[:, 0:NCHUNK, 3:5],
                        lnS[:, 0:8].rearrange("p (j w) -> p j w", j=NCHUNK),
                        1.0 / T,
                        LSEB,
                        op0=ALU.mult,
                        op1=ALU.add,
                    )
                    nc.vector.tensor_scalar(
                        maxv[:, 2:NCHUNK, 2:3],
                        lnS[:, 8:10].rearrange("p (j w) -> p j w", j=2),
                        1.0 / T,
                        LSEB,
                        op0=ALU.mult,
                        op1=ALU.add,
                    )

                    # logits: three accumulation groups in misc cols 4:7:
                    #   col 4: classes {0,1} raw * invq
                    #   col 5: classes {3,4} LSE * valid-row mask
                    #   col 6: class 2 (raw * invq j<2, LSE * mask j>=2)
                    LG = NCHUNK
                    n_mm = 3 * NCHUNK
                    k = 0
                    for j in range(NCHUNK):
                        nc.tensor.matmul(
                            misc[0:2, LG : LG + 1],
                            lhsT=maxv[:, j, 0:2],
                            rhs=inv[:, j : j + 1],
                            start=(k == 0),
                            stop=(k == n_mm - 1),
                            skip_group_check=True,
                        )
                        k += 1
                        nc.tensor.matmul(
                            misc[0:2, LG + 1 : LG + 2],
                            lhsT=maxv[:, j, 3:5],
                            rhs=maskb[:, j : j + 1],
                            start=False,
                            stop=(k == n_mm - 1),
                            skip_group_check=True,
                        )
                        k += 1
                        nc.tensor.matmul(
                            misc[0:1, LG + 2 : LG + 3],
                            lhsT=maxv[:, j, 2:3],
                            rhs=(inv if j < 2 else maskb)[:, j : j + 1],
                            start=False,
                            stop=(k == n_mm - 1),
                            skip_group_check=True,
                        )
                        k += 1
                    nc.vector.tensor_copy(stage3[:, :, i], misc[0:2, LG : LG + 3])

                # transpose the staged logits to query-major so the final
                # DMA writes contiguous DRAM rows (avoids per-element descs)
                outPS = miscps.tile([QPC, NWAY], F32, tag="misc")
                nc.tensor.matmul(
                    outPS[:, 0:2],
                    lhsT=stage3[:, 0, :],
                    rhs=ident[0:2, 0:2],
                    is_transpose=True,
                    start=True,
                    stop=True,
                )
                nc.tensor.matmul(
                    outPS[:, 3:5],
                    lhsT=stage3[:, 1, :],
                    rhs=ident[0:2, 0:2],
                    is_transpose=True,
                    start=True,
                    stop=True,
                )
                nc.tensor.matmul(
                    outPS[:, 2:3],
                    lhsT=stage3[0:1, 2, :],
                    rhs=ident[0:1, 0:1],
                    is_transpose=True,
                    start=True,
                    stop=True,
                )
                stageF = cpool.tile([QPC, NWAY], F32)
                nc.vector.tensor_copy(stageF[:], outPS[:])
            nc.sync.dma_start(out_d[:], stageF[:])

    nc.compile()
    return nc


def _get_program():
    if "nc" not in _CACHE:
        _CACHE["nc"] = _build_program()
    return _CACHE["nc"]


def _make_in_maps(support_xf, query_xf):
    sup = np.ascontiguousarray(np.asarray(support_xf, dtype=np.float32)).reshape(
        B, NWAY * KSHOT, C, HW
    )
    qry = np.ascontiguousarray(np.asarray(query_xf, dtype=np.float32)).reshape(B, Q, C, HW)
    idn = np.eye(128, dtype=np.float32)
    msk = np.zeros((128, NCHUNK), dtype=np.float32)
    for j in range(NCHUNK):
        lo, hi = _chunk_cols(j)
        msk[0 : hi - lo, j] = 1.0
    in_maps = []
    spans = []
    for core in range(8):
        bi = core // 4
        lo = (core % 4) * QPC
        hi = min(lo + QPC, Q)
        qs = qry[bi, lo:hi]
        if hi - lo < QPC:
            pad = np.repeat(qs[-1:], QPC - (hi - lo), axis=0)
            qs = np.concatenate([qs, pad], axis=0)
        in_maps.append(
            {
                "sup": np.ascontiguousarray(sup[bi]),
                "qry": np.ascontiguousarray(qs),
                "idn": idn,
                "msk": msk,
            }
        )
        spans.append((bi, lo, hi))
    return in_maps, spans


def _run(in_maps, **kwargs):
    nc = _get_program()
    return run_bass_kernel_spmd(nc, in_maps, list(range(8)), **kwargs)


def kernel(support_xf, support_y, query_xf, query_y, n_way=NWAY, k_shot=KSHOT, **_):
    in_maps, spans = _make_in_maps(support_xf, query_xf)
    res = _run(in_maps)
    logits = np.zeros((B * Q, NWAY), dtype=np.float32)
    for core, (bi, lo, hi) in enumerate(spans):
        logits[bi * Q + lo : bi * Q + hi] = res.results[core]["out"][: hi - lo]
    return logits
